# revision 6
# baseline (speedup 1.0000x reference)
"""Trainium2 Bass kernel for nn_Attention_12463995093474 (sparse_attention).

Math (reference):
  q/k/v = content linears; 2 absolute heads, 2 relative heads (DK=32).
  abs:  scores = (Xq_a + abs_kernel@abs_q_w) @ (Xk_a + abs_kernel@abs_k_w)^T
  rel:  scores = Xq_r @ Xk_r^T + (Xq_r + rel_bias) . (rel_kernel@rel_k_w + rel_k_b)
  softmax(mask) @ v -> out linear.

Key optimization: the dominant term
    sum_o (Xq_r+rel_bias)[i,o] * (sum_d rel_kernel[i,j,d] rel_k_w[d,o] + rel_k_b[o])
is reassociated to
    sum_d rel_kernel[i,j,d] * u[i,d] + c[i],
      u = rel_k_w @ (Xq_r+rel_bias)^T,  c = (Xq_r+rel_bias) @ rel_k_b
which turns a 21-GFLOP tensor contraction into a single streaming pass over
rel_kernel (655 MB) in bf16: elementwise multiply by u (free-dim-broadcast AP)
then a pairwise tree reduction over d.

v2 changes vs baseline:
  - streaming chunks split between DVE and Pool (gpsimd) engines (~60/40),
    with the rel_kernel DMAs moved to the HWDGE queues (sync/scalar) so the
    Q7 cores are free to run Pool tensor ops.
  - score assembly fused with scalar_tensor_tensor; p kept unnormalized in
    bf16; 1/rowsum folded into the per-head ACT copies of x out of PSUM.
  - output projection computes y = x @ Wo directly (one transpose saved).

Sharding: data-parallel over batch, B=16 -> 2 batches per core on 8 cores.
"""

import numpy as np
from contextlib import ExitStack

import concourse.bass as bass
import concourse.bacc as bacc
import concourse.tile as tile
from concourse import mybir
from concourse.masks import make_identity
from concourse.bass_utils import run_bass_kernel_spmd

B, T, D = 16, 200, 128
H_ABS, H_REL, H, DK = 2, 2, 4, 32
N_CORES = 8
BPC = B // N_CORES  # batches per core
SCALE = 1.0 / float(DK) ** 0.5
JC = 50  # j-chunk for the streaming pass (T % JC == 0)
TT = BPC * T  # tokens per core (400)

F32 = mybir.dt.float32
BF16 = mybir.dt.bfloat16
I32 = mybir.dt.int32
AX = mybir.AxisListType
OP = mybir.AluOpType
AF = mybir.ActivationFunctionType

# i-blocks per batch: (start, len)
IBLOCKS = [(0, 128), (128, T - 128)]

# of each round of 32 streaming chunks, how many go to the Pool engine
POOL_CHUNKS = 13
N_CHUNKS = H_REL * BPC * (T // JC) * len(IBLOCKS)  # 32


def chunk_engine_plan():
    """Boolean list over chunk index: True -> Pool, False -> DVE."""
    plan = []
    acc = 0
    for i in range(N_CHUNKS):
        acc += POOL_CHUNKS
        if acc >= N_CHUNKS:
            acc -= N_CHUNKS
            plan.append(True)
        else:
            plan.append(False)
    return plan


def build_kernel(ctx: ExitStack, tc: tile.TileContext, io: dict):
    nc = tc.nc

    query = io["query"].flatten_outer_dims()  # [400, 128]
    key = io["key"].flatten_outer_dims()
    value = io["value"].flatten_outer_dims()
    mask = io["mask"]          # [2, 1, 200, 200] i32
    relk = io["rel_kernel"]    # [2, 2, 200, 200, 128] bf16
    absk = io["abs_kernel"]    # [2, 2, 200, 128]
    out = io["out"]            # [2, 200, 128]

    consts = ctx.enter_context(tc.tile_pool(name="consts", bufs=1))
    prep = ctx.enter_context(tc.tile_pool(name="prep", bufs=2))
    keep = ctx.enter_context(tc.tile_pool(name="keep", bufs=1))

    prep_dmas = []  # all prep dma instructions (gate for the big stream)
    _eng = [0]

    def small_dma(out_ap, in_ap):
        # alternate the two HWDGE sequencers so issue isn't serialized
        eng = nc.sync if _eng[0] % 2 == 0 else nc.scalar
        _eng[0] += 1
        i = eng.dma_start(out_ap, in_ap)
        prep_dmas.append(i)
        return i

    ident = consts.tile([128, 128], F32, tag="ident")
    make_identity(nc, ident)
    identb = consts.tile([128, 128], BF16, tag="identb")
    nc.gpsimd.tensor_copy(identb, ident)

    def load_const(name, ap, shape):
        t = consts.tile(shape, F32, tag=name)
        small_dma(t, ap)
        return t

    with tc.tile_pool(name="psum_prep", bufs=2, space="PSUM") as psp, \
         tc.tile_pool(name="psum_prep1", bufs=2, space="PSUM") as psp1:

        # transpose token-major inputs to [din, t]
        def transpose_in(src_ap, tag):
            xt = keep.tile([128, TT], F32, tag=tag)
            for ti, t0 in enumerate(range(0, TT, 128)):
                tl = min(128, TT - t0)
                raw = prep.tile([128, 128], F32, tag="t_raw")
                small_dma(raw[:tl, :], src_ap[t0 : t0 + tl, :])
                tp = psp.tile([128, 128], F32, tag="t_ps")
                nc.tensor.transpose(tp[:, :tl], raw[:tl, :], ident[:tl, :tl])
                nc.scalar.copy(xt[:, t0 : t0 + tl], tp[:, :tl])
            return xt

        # ---- critical path first: everything the streaming pass needs ----
        xqT = transpose_in(query, "xqT")

        wq = load_const("wq", io["Wq"], [128, 128])
        bq_c = load_const("bq", io["bq"], [128, 1])
        bq_s = consts.tile([128, 1], F32, tag="bq_s")
        nc.scalar.activation(bq_s, bq_c, AF.Copy, scale=SCALE)

        rkw = {}
        small_cols = {}
        for hr in range(H_REL):
            rkw[hr] = load_const(f"rkw{hr}", io["rel_k_w"][hr], [128, DK])
            small_cols[("rkb", hr)] = load_const(
                f"rkb{hr}", io["rel_k_b"][hr], [DK, 1])
            t = load_const(f"rbias{hr}", io["rel_bias"][0, hr, 0, :], [DK, 1])
            ts_ = consts.tile([DK, 1], F32, tag=f"rbias_s{hr}")
            nc.scalar.activation(ts_, t, AF.Copy, scale=SCALE)
            small_cols[("rbias_s", hr)] = ts_

        rkwT = {}
        for hr in range(H_REL):
            tp = psp.tile([DK, 128], F32, tag="mm_ps")
            nc.tensor.transpose(tp, rkw[hr], ident)
            t = keep.tile([DK, 128], F32, tag=f"rkwT{hr}")
            nc.scalar.copy(t, tp)
            rkwT[hr] = t

        qT = {}
        for h in (H_ABS, H_ABS + 1, 0, 1):  # rel heads first
            qp = psp.tile([DK, TT], F32, tag="mm_ps")
            nc.tensor.matmul(qp, wq[:, DK * h : DK * (h + 1)], xqT)
            t = keep.tile([DK, TT], F32, tag=f"qT{h}")
            nc.scalar.activation(t, qp, AF.Identity,
                                 bias=bq_s[DK * h : DK * (h + 1)], scale=SCALE)
            qT[h] = t

        qrbT = {}
        for hr in range(H_REL):
            t = keep.tile([DK, TT], F32, tag=f"qrbT{hr}")
            nc.vector.tensor_scalar(t, qT[H_ABS + hr],
                                    small_cols[("rbias_s", hr)], None, OP.add)
            qrbT[hr] = t

        u_b = {}
        c_sb = {}
        for hr in range(H_REL):
            for b in range(BPC):
                for ib, (i0, il) in enumerate(IBLOCKS):
                    tsl = slice(b * T + i0, b * T + i0 + il)
                    up = psp1.tile([128, 128], F32, tag="sm_ps")
                    nc.tensor.matmul(up[:il, :], qrbT[hr][:, tsl], rkwT[hr])
                    t = keep.tile([128, 128], BF16, tag=f"ub{hr}_{b}_{ib}")
                    nc.scalar.copy(t[:il, :], up[:il, :])
                    u_b[(hr, b, ib)] = t

                    cp = psp1.tile([128, 1], F32, tag="sm_ps")
                    nc.tensor.matmul(cp[:il, :], qrbT[hr][:, tsl],
                                     small_cols[("rkb", hr)])
                    t = keep.tile([128, 1], F32, tag=f"c{hr}_{b}_{ib}")
                    nc.scalar.copy(t[:il, :], cp[:il, :])
                    c_sb[(hr, b, ib)] = t

        # ---- rest of prep ----
        xkT = transpose_in(key, "xkT")
        xvT = transpose_in(value, "xvT")

        wk = load_const("wk", io["Wk"], [128, 128])
        wv = load_const("wv", io["Wv"], [128, 128])
        wo = load_const("wo", io["Wo"], [128, 128])
        bk_c = load_const("bk", io["bk"], [128, 1])
        bv_b = consts.tile([128, 128], F32, tag="bv_b")
        bv_ap = io["bv"]
        small_dma(bv_b, bass.AP(tensor=bv_ap.tensor, offset=bv_ap.offset,
                                ap=[[0, 128]] + bv_ap.ap))
        bo_b = consts.tile([128, 128], F32, tag="bo_b")
        bo_ap = io["bo"]
        small_dma(bo_b, bass.AP(tensor=bo_ap.tensor, offset=bo_ap.offset,
                                ap=[[0, 128]] + bo_ap.ap))

        abs_w = {}
        for hh in range(H_ABS):
            abs_w[("aqw", hh)] = load_const(f"aqw{hh}", io["abs_q_w"][hh], [128, DK])
            abs_w[("akw", hh)] = load_const(f"akw{hh}", io["abs_k_w"][hh], [128, DK])
            small_cols[("akb", hh)] = load_const(
                f"akb{hh}", io["abs_k_b"][hh], [DK, 1])
            t = load_const(f"aqb{hh}", io["abs_q_b"][hh], [DK, 1])
            ts_ = consts.tile([DK, 1], F32, tag=f"aqb_s{hh}")
            nc.scalar.activation(ts_, t, AF.Copy, scale=SCALE)
            small_cols[("aqb_s", hh)] = ts_

        kT = {}
        for h in range(H):
            kp = psp.tile([DK, TT], F32, tag="mm_ps")
            nc.tensor.matmul(kp, wk[:, DK * h : DK * (h + 1)], xkT)
            t = keep.tile([DK, TT], F32, tag=f"kT{h}")
            nc.scalar.activation(t, kp, AF.Identity,
                                 bias=bk_c[DK * h : DK * (h + 1)])
            kT[h] = t

        vb = {}
        for b in range(BPC):
            for jb, (j0, jl) in enumerate(IBLOCKS):
                vp = psp1.tile([128, 128], F32, tag="sm_ps")
                nc.tensor.matmul(vp[:jl, :], xvT[:, b * T + j0 : b * T + j0 + jl], wv)
                t = keep.tile([128, 128], BF16, tag=f"v{b}_{jb}")
                nc.vector.tensor_add(t[:jl, :], vp[:jl, :], bv_b[:jl, :])
                vb[(b, jb)] = t

        qaT = {}
        kaT = {}
        for hh in range(H_ABS):
            akT = transpose_in(absk[hh].flatten_outer_dims(), f"akT{hh}")
            pp = psp.tile([DK, TT], F32, tag="mm_ps")
            nc.tensor.matmul(pp, abs_w[("aqw", hh)], akT)
            pqT = prep.tile([DK, TT], F32, tag="pqT")
            nc.scalar.activation(pqT, pp, AF.Identity,
                                 bias=small_cols[("aqb_s", hh)], scale=SCALE)
            t = keep.tile([DK, TT], F32, tag=f"qaT{hh}")
            nc.vector.tensor_add(t, qT[hh], pqT)
            qaT[hh] = t

            pp2 = psp.tile([DK, TT], F32, tag="mm_ps")
            nc.tensor.matmul(pp2, abs_w[("akw", hh)], akT)
            pkT = prep.tile([DK, TT], F32, tag="pqT")
            nc.scalar.activation(pkT, pp2, AF.Identity,
                                 bias=small_cols[("akb", hh)])
            t = keep.tile([DK, TT], F32, tag=f"kaT{hh}")
            nc.vector.tensor_add(t, kT[hh], pkT)
            kaT[hh] = t

        mb = {}
        for b in range(BPC):
            for ib, (i0, il) in enumerate(IBLOCKS):
                mi = prep.tile([128, T], I32, tag="m_i32")
                small_dma(mi[:il, :], mask[b, 0, i0 : i0 + il, :])
                t = keep.tile([128, T], F32, tag=f"mb{b}_{ib}")
                nc.vector.tensor_scalar(t[:il, :], mi[:il, :], 1e9, -1e9,
                                        OP.mult, OP.add)
                mb[(b, ib)] = t

    # funnel: fires when every prep DMA has completed; gates the big stream
    funnel_t = consts.tile([1, 1], F32, tag="funnel")
    funnel = nc.vector.memset(funnel_t, 0.0)
    for di in prep_dmas:
        bass._add_dep_helper(funnel.ins, di.ins, True, "prep dma done")

    # ---------------- main phase ----------------
    stream = ctx.enter_context(tc.tile_pool(name="stream", bufs=5))
    wpool = ctx.enter_context(tc.tile_pool(name="wpool", bufs=3))
    tree = ctx.enter_context(tc.tile_pool(name="tree", bufs=2))
    s2pool = ctx.enter_context(tc.tile_pool(name="s2pool", bufs=2))
    sm = ctx.enter_context(tc.tile_pool(name="sm", bufs=2))
    ps_s1 = ctx.enter_context(tc.tile_pool(name="ps_s1", bufs=2, space="PSUM"))
    ps_tp = ctx.enter_context(tc.tile_pool(name="ps_tp", bufs=2, space="PSUM"))
    ps_tail = ctx.enter_context(tc.tile_pool(name="ps_tail", bufs=1, space="PSUM"))

    n_gated = [0]
    plan = chunk_engine_plan()
    chunk_idx = [0]

    def stream_chunk(hr, b, ib, jc0, s2t, ub):
        i0, il = IBLOCKS[ib]
        use_pool = plan[chunk_idx[0] % N_CHUNKS]
        chunk_idx[0] += 1
        eng = nc.gpsimd if use_pool else nc.vector
        wp = wpool
        tp = tree

        rk = stream.tile([128, JC, 128], BF16, tag="rk")
        # HWDGE stream: alternate the two hardware DGE rings
        dma_eng = nc.sync if chunk_idx[0] % 2 == 0 else nc.scalar
        dma_i = dma_eng.dma_start(
            rk[:il], relk[hr, b, i0 : i0 + il, jc0 : jc0 + JC, :]
        )
        if n_gated[0] < 12:
            bass._add_dep_helper(dma_i.ins, funnel.ins, True,
                                 "hold stream until prep loads done")
            n_gated[0] += 1
        w = wp.tile([128, JC, 128], BF16, tag="w")
        eng.tensor_tensor(
            w[:il], rk[:il],
            ub[:il, :].unsqueeze(1).broadcast_to([il, JC, 128]),
            op=OP.mult,
        )
        cur = w
        width = 64
        while width >= 2:
            nxt = tp.tile([128, JC, width], BF16, tag=f"L{width}")
            eng.tensor_add(
                nxt[:il], cur[:il, :, 0:width], cur[:il, :, width : 2 * width]
            )
            cur = nxt
            width //= 2
        eng.tensor_add(
            s2t[:il, jc0 : jc0 + JC], cur[:il, :, 0], cur[:il, :, 1]
        )

    for b in range(BPC):
        # stream both i-blocks interleaved: their DMAs hit disjoint
        # partition ranges -> more SDMA engines active concurrently
        s2 = {0: {}, 1: {}}
        for hr in range(H_REL):
            for ib in range(2):
                s2[ib][hr] = s2pool.tile([128, T], F32, tag=f"s2_{hr}_{ib}", name=f"s2_{hr}_{ib}")
            for jc0 in range(0, T, JC):
                for ib in range(2):
                    stream_chunk(hr, b, ib, jc0, s2[ib][hr], u_b[(hr, b, ib)])

        x_ps = {ib: ps_tail.tile([128, 128], F32, tag=f"x{ib}", name=f"x{ib}")
                for ib in range(2)}
        rcps = {}
        for ib, (i0, il) in enumerate(IBLOCKS):
            tsl = slice(b * T + i0, b * T + i0 + il)
            for h in range(H):
                is_rel = h >= H_ABS
                lhs = qaT[h] if not is_rel else qT[h]
                rhs = kaT[h] if not is_rel else kT[h]
                s1 = ps_s1.tile([128, T], F32, tag="s1")
                nc.tensor.matmul(s1[:il, :], lhs[:, tsl],
                                 rhs[:, b * T : (b + 1) * T])

                st = sm.tile([128, T], F32, tag="st")
                if is_rel:
                    hr = h - H_ABS
                    # st = (s1 + c) + s2, then + mask
                    nc.vector.scalar_tensor_tensor(
                        st[:il, :], s1[:il, :], c_sb[(hr, b, ib)][:il],
                        s2[ib][hr][:il, :], op0=OP.add, op1=OP.add)
                    nc.vector.tensor_add(st[:il, :], st[:il, :],
                                         mb[(b, ib)][:il, :])
                else:
                    nc.vector.tensor_add(st[:il, :], s1[:il, :],
                                         mb[(b, ib)][:il, :])

                nmax = sm.tile([128, 1], F32, tag="nmax")
                nc.vector.tensor_reduce(nmax[:il], st[:il, :], AX.X, OP.max,
                                        negate=True)
                # unnormalized p in bf16; 1/rsum folded into x copy later
                p = sm.tile([128, T], BF16, tag="p")
                rsum = sm.tile([128, 1], F32, tag="rsum")
                nc.scalar.activation(p[:il, :], st[:il, :], AF.Exp,
                                     bias=nmax[:il], accum_out=rsum[:il])
                rcp = sm.tile([128, 1], F32, tag=f"rcp{ib}_{h}")
                nc.vector.reciprocal(rcp[:il], rsum[:il])
                rcps[(ib, h)] = rcp

                hsl = slice(DK * h, DK * (h + 1))
                for jb, (j0, jl) in enumerate(IBLOCKS):
                    tp = ps_tp.tile([128, 128], BF16, tag="tp")
                    nc.tensor.transpose(tp[:jl, :il], p[:il, j0 : j0 + jl],
                                        identb[:il, :il])
                    pT = sm.tile([128, 128], BF16, tag="pT")
                    nc.scalar.copy(pT[:jl, :il], tp[:jl, :il])
                    nc.tensor.matmul(x_ps[ib][:il, hsl], pT[:jl, :il],
                                     vb[(b, jb)][:jl, hsl],
                                     start=(jb == 0), stop=(jb == 1))

        for ib, (i0, il) in enumerate(IBLOCKS):
            x_sb = sm.tile([128, 128], F32, tag="x_sb")
            for h in range(H):
                hsl = slice(DK * h, DK * (h + 1))
                nc.scalar.activation(x_sb[:il, hsl], x_ps[ib][:il, hsl],
                                     AF.Copy, scale=rcps[(ib, h)][:il])
            xT_ps = ps_tail.tile([128, 128], F32, tag="tail3")
            nc.tensor.transpose(xT_ps[:, :il], x_sb[:il, :], ident[:il, :il])
            xT_sb = sm.tile([128, 128], F32, tag="xT_sb")
            nc.scalar.copy(xT_sb[:, :il], xT_ps[:, :il])
            y_ps = ps_tail.tile([128, 128], F32, tag="tail3")
            nc.tensor.matmul(y_ps[:il, :], xT_sb[:, :il], wo)
            y_sb = sm.tile([128, 128], F32, tag="y_sb")
            nc.vector.tensor_add(y_sb[:il, :], y_ps[:il, :], bo_b[:il, :])
            nc.sync.dma_start(out[b, i0 : i0 + il, :], y_sb[:il, :])


def build_nc():
    nc = bacc.Bacc(trn_type="TRN2")
    io = {}
    io["query"] = nc.dram_tensor("query", [BPC, T, D], F32, kind="ExternalInput").ap()
    io["key"] = nc.dram_tensor("key", [BPC, T, D], F32, kind="ExternalInput").ap()
    io["value"] = nc.dram_tensor("value", [BPC, T, D], F32, kind="ExternalInput").ap()
    io["mask"] = nc.dram_tensor("mask", [BPC, 1, T, T], I32, kind="ExternalInput").ap()
    io["rel_kernel"] = nc.dram_tensor(
        "rel_kernel", [H_REL, BPC, T, T, D], BF16, kind="ExternalInput"
    ).ap()
    io["abs_kernel"] = nc.dram_tensor(
        "abs_kernel", [H_ABS, BPC, T, D], F32, kind="ExternalInput"
    ).ap()
    for nm, shape in [
        ("Wq", [D, D]), ("bq", [D]), ("Wk", [D, D]), ("bk", [D]),
        ("Wv", [D, D]), ("bv", [D]),
        ("abs_q_w", [H_ABS, D, DK]), ("abs_q_b", [H_ABS, DK]),
        ("abs_k_w", [H_ABS, D, DK]), ("abs_k_b", [H_ABS, DK]),
        ("rel_k_w", [H_REL, D, DK]), ("rel_k_b", [H_REL, DK]),
        ("rel_bias", [1, H_REL, 1, DK]),
        ("Wo", [D, D]), ("bo", [D]),
    ]:
        io[nm] = nc.dram_tensor(nm, shape, F32, kind="ExternalInput").ap()
    io["out"] = nc.dram_tensor("out", [BPC, T, D], F32, kind="ExternalOutput").ap()

    with tile.TileContext(nc) as tc:
        with ExitStack() as ctx:
            build_kernel(ctx, tc, io)
    nc.compile()
    return nc


_NC_CACHE = None


def _get_nc():
    global _NC_CACHE
    if _NC_CACHE is None:
        _NC_CACHE = build_nc()
    return _NC_CACHE


def make_in_maps(inputs):
    """Shard full inputs into per-core input maps."""
    f32 = np.float32
    weights = {
        nm: np.ascontiguousarray(np.asarray(inputs[nm], dtype=f32))
        for nm in ["Wq", "bq", "Wk", "bk", "Wv", "bv", "abs_q_w", "abs_q_b",
                   "abs_k_w", "abs_k_b", "rel_k_w", "rel_k_b", "rel_bias",
                   "Wo", "bo"]
    }
    query = np.asarray(inputs["query"], dtype=f32)
    key = np.asarray(inputs["key"], dtype=f32)
    value = np.asarray(inputs["value"], dtype=f32)
    mask = np.asarray(inputs["mask"], dtype=np.int32)
    import ml_dtypes
    relk = np.asarray(inputs["rel_kernel"], dtype=f32).astype(ml_dtypes.bfloat16)
    absk = np.asarray(inputs["abs_kernel"], dtype=f32)

    in_maps = []
    for c in range(N_CORES):
        bs = slice(c * BPC, (c + 1) * BPC)
        m = dict(weights)
        m["query"] = np.ascontiguousarray(query[bs])
        m["key"] = np.ascontiguousarray(key[bs])
        m["value"] = np.ascontiguousarray(value[bs])
        m["mask"] = np.ascontiguousarray(mask[bs])
        m["rel_kernel"] = np.ascontiguousarray(relk[:, bs])
        m["abs_kernel"] = np.ascontiguousarray(absk[:, bs])
        in_maps.append(m)
    return in_maps


def kernel(**inputs) -> np.ndarray:
    nc = _get_nc()
    in_maps = make_in_maps(inputs)
    res = run_bass_kernel_spmd(nc, in_maps, core_ids=list(range(N_CORES)))
    return np.concatenate([r["out"] for r in res.results], axis=0)


if __name__ == "__main__":
    nc = build_nc()
    print("built ok")


# revision 7
# speedup vs baseline: 1.6464x; 1.6464x over previous
"""Trainium2 Bass kernel for nn_Attention_12463995093474 (sparse_attention).

Math (reference):
  q/k/v = content linears; 2 absolute heads, 2 relative heads (DK=32).
  abs:  scores = (Xq_a + abs_kernel@abs_q_w) @ (Xk_a + abs_kernel@abs_k_w)^T
  rel:  scores = Xq_r @ Xk_r^T + (Xq_r + rel_bias) . (rel_kernel@rel_k_w + rel_k_b)
  softmax(mask) @ v -> out linear.

Key optimization: the dominant term
    sum_o (Xq_r+rel_bias)[i,o] * (sum_d rel_kernel[i,j,d] rel_k_w[d,o] + rel_k_b[o])
is reassociated to
    sum_d rel_kernel[i,j,d] * u[i,d] + c[i],
      u = rel_k_w @ (Xq_r+rel_bias)^T,  c = (Xq_r+rel_bias) @ rel_k_b
which turns a 21-GFLOP tensor contraction into a single streaming pass over
rel_kernel (655 MB) in bf16: elementwise multiply by u (free-dim-broadcast AP)
then a pairwise tree reduction over d.

v2 changes vs baseline:
  - streaming chunks split between DVE and Pool (gpsimd) engines (~60/40),
    with the rel_kernel DMAs moved to the HWDGE queues (sync/scalar) so the
    Q7 cores are free to run Pool tensor ops.
  - score assembly fused with scalar_tensor_tensor; p kept unnormalized in
    bf16; 1/rowsum folded into the per-head ACT copies of x out of PSUM.
  - output projection computes y = x @ Wo directly (one transpose saved).

Sharding: data-parallel over batch, B=16 -> 2 batches per core on 8 cores.
"""

import numpy as np
from contextlib import ExitStack

import concourse.bass as bass
import concourse.bacc as bacc
import concourse.tile as tile
from concourse import mybir
from concourse.masks import make_identity
from concourse.bass_utils import run_bass_kernel_spmd

B, T, D = 16, 200, 128
H_ABS, H_REL, H, DK = 2, 2, 4, 32
N_CORES = 8
BPC = B // N_CORES  # batches per core
SCALE = 1.0 / float(DK) ** 0.5
JC = 50  # j-chunk for the streaming pass (T % JC == 0)
TT = BPC * T  # tokens per core (400)

F32 = mybir.dt.float32
BF16 = mybir.dt.bfloat16
I32 = mybir.dt.int32
AX = mybir.AxisListType
OP = mybir.AluOpType
AF = mybir.ActivationFunctionType

# i-blocks per batch: (start, len)
IBLOCKS = [(0, 128), (128, T - 128)]

# of each round of 32 streaming chunks, how many go to the Pool engine.
# NOTE: DVE tensor_tensor needs the SBUF port pair that GpSimd also uses
# (exclusive per-instruction lock), so Pool chunks serialize against DVE
# chunks -- keep this at 0.
POOL_CHUNKS = 0
N_CHUNKS = H_REL * BPC * (T // JC) * len(IBLOCKS)  # 32


def chunk_engine_plan():
    """Boolean list over chunk index: True -> Pool, False -> DVE."""
    plan = []
    acc = 0
    for i in range(N_CHUNKS):
        acc += POOL_CHUNKS
        if acc >= N_CHUNKS:
            acc -= N_CHUNKS
            plan.append(True)
        else:
            plan.append(False)
    return plan


def build_kernel(ctx: ExitStack, tc: tile.TileContext, io: dict):
    nc = tc.nc

    query = io["query"].flatten_outer_dims()  # [400, 128]
    key = io["key"].flatten_outer_dims()
    value = io["value"].flatten_outer_dims()
    mask = io["mask"]          # [2, 1, 200, 200] i32
    relk = io["rel_kernel"]    # [2, 2, 200, 200, 128] bf16
    absk = io["abs_kernel"]    # [2, 2, 200, 128]
    out = io["out"]            # [2, 200, 128]

    consts = ctx.enter_context(tc.tile_pool(name="consts", bufs=1))
    prep = ctx.enter_context(tc.tile_pool(name="prep", bufs=2))
    keep = ctx.enter_context(tc.tile_pool(name="keep", bufs=1))

    prep_dmas = []  # all prep dma instructions (gate for the big stream)
    _eng = [0]

    def small_dma(out_ap, in_ap):
        # alternate the two HWDGE sequencers so issue isn't serialized
        eng = nc.sync if _eng[0] % 2 == 0 else nc.scalar
        _eng[0] += 1
        i = eng.dma_start(out_ap, in_ap)
        prep_dmas.append(i)
        return i

    ident = consts.tile([128, 128], F32, tag="ident")
    make_identity(nc, ident)
    identb = consts.tile([128, 128], BF16, tag="identb")
    nc.gpsimd.tensor_copy(identb, ident)

    def load_const(name, ap, shape):
        t = consts.tile(shape, F32, tag=name)
        small_dma(t, ap)
        return t

    with tc.tile_pool(name="psum_prep", bufs=2, space="PSUM") as psp, \
         tc.tile_pool(name="psum_prep1", bufs=2, space="PSUM") as psp1:

        # transpose token-major inputs to [din, t]
        def transpose_in(src_ap, tag):
            xt = keep.tile([128, TT], F32, tag=tag)
            for ti, t0 in enumerate(range(0, TT, 128)):
                tl = min(128, TT - t0)
                raw = prep.tile([128, 128], F32, tag="t_raw")
                small_dma(raw[:tl, :], src_ap[t0 : t0 + tl, :])
                tp = psp.tile([128, 128], F32, tag="t_ps")
                nc.tensor.transpose(tp[:, :tl], raw[:tl, :], ident[:tl, :tl])
                nc.scalar.copy(xt[:, t0 : t0 + tl], tp[:, :tl])
            return xt

        # ---- critical path first: everything the streaming pass needs ----
        xqT = transpose_in(query, "xqT")

        wq = load_const("wq", io["Wq"], [128, 128])
        bq_c = load_const("bq", io["bq"], [128, 1])
        bq_s = consts.tile([128, 1], F32, tag="bq_s")
        nc.scalar.activation(bq_s, bq_c, AF.Copy, scale=SCALE)

        rkw = {}
        small_cols = {}
        for hr in range(H_REL):
            rkw[hr] = load_const(f"rkw{hr}", io["rel_k_w"][hr], [128, DK])
            small_cols[("rkb", hr)] = load_const(
                f"rkb{hr}", io["rel_k_b"][hr], [DK, 1])
            t = load_const(f"rbias{hr}", io["rel_bias"][0, hr, 0, :], [DK, 1])
            ts_ = consts.tile([DK, 1], F32, tag=f"rbias_s{hr}")
            nc.scalar.activation(ts_, t, AF.Copy, scale=SCALE)
            small_cols[("rbias_s", hr)] = ts_

        rkwT = {}
        for hr in range(H_REL):
            tp = psp.tile([DK, 128], F32, tag="mm_ps")
            nc.tensor.transpose(tp, rkw[hr], ident)
            t = keep.tile([DK, 128], F32, tag=f"rkwT{hr}")
            nc.scalar.copy(t, tp)
            rkwT[hr] = t

        qT = {}
        for h in (H_ABS, H_ABS + 1, 0, 1):  # rel heads first
            qp = psp.tile([DK, TT], F32, tag="mm_ps")
            nc.tensor.matmul(qp, wq[:, DK * h : DK * (h + 1)], xqT)
            t = keep.tile([DK, TT], F32, tag=f"qT{h}")
            nc.scalar.activation(t, qp, AF.Identity,
                                 bias=bq_s[DK * h : DK * (h + 1)], scale=SCALE)
            qT[h] = t

        qrbT = {}
        for hr in range(H_REL):
            t = keep.tile([DK, TT], F32, tag=f"qrbT{hr}")
            nc.vector.tensor_scalar(t, qT[H_ABS + hr],
                                    small_cols[("rbias_s", hr)], None, OP.add)
            qrbT[hr] = t

        u_b = {}
        c_sb = {}
        for hr in range(H_REL):
            for b in range(BPC):
                for ib, (i0, il) in enumerate(IBLOCKS):
                    tsl = slice(b * T + i0, b * T + i0 + il)
                    up = psp1.tile([128, 128], F32, tag="sm_ps")
                    nc.tensor.matmul(up[:il, :], qrbT[hr][:, tsl], rkwT[hr])
                    t = keep.tile([128, 128], BF16, tag=f"ub{hr}_{b}_{ib}")
                    nc.scalar.copy(t[:il, :], up[:il, :])
                    u_b[(hr, b, ib)] = t

                    cp = psp1.tile([128, 1], F32, tag="sm_ps")
                    nc.tensor.matmul(cp[:il, :], qrbT[hr][:, tsl],
                                     small_cols[("rkb", hr)])
                    t = keep.tile([128, 1], F32, tag=f"c{hr}_{b}_{ib}")
                    nc.scalar.copy(t[:il, :], cp[:il, :])
                    c_sb[(hr, b, ib)] = t

        # ---- rest of prep ----
        xkT = transpose_in(key, "xkT")
        xvT = transpose_in(value, "xvT")

        wk = load_const("wk", io["Wk"], [128, 128])
        wv = load_const("wv", io["Wv"], [128, 128])
        wo = load_const("wo", io["Wo"], [128, 128])
        bk_c = load_const("bk", io["bk"], [128, 1])
        bv_b = consts.tile([128, 128], F32, tag="bv_b")
        bv_ap = io["bv"]
        small_dma(bv_b, bass.AP(tensor=bv_ap.tensor, offset=bv_ap.offset,
                                ap=[[0, 128]] + bv_ap.ap))
        bo_b = consts.tile([128, 128], F32, tag="bo_b")
        bo_ap = io["bo"]
        small_dma(bo_b, bass.AP(tensor=bo_ap.tensor, offset=bo_ap.offset,
                                ap=[[0, 128]] + bo_ap.ap))

        abs_w = {}
        for hh in range(H_ABS):
            abs_w[("aqw", hh)] = load_const(f"aqw{hh}", io["abs_q_w"][hh], [128, DK])
            abs_w[("akw", hh)] = load_const(f"akw{hh}", io["abs_k_w"][hh], [128, DK])
            small_cols[("akb", hh)] = load_const(
                f"akb{hh}", io["abs_k_b"][hh], [DK, 1])
            t = load_const(f"aqb{hh}", io["abs_q_b"][hh], [DK, 1])
            ts_ = consts.tile([DK, 1], F32, tag=f"aqb_s{hh}")
            nc.scalar.activation(ts_, t, AF.Copy, scale=SCALE)
            small_cols[("aqb_s", hh)] = ts_

        kT = {}
        for h in range(H):
            kp = psp.tile([DK, TT], F32, tag="mm_ps")
            nc.tensor.matmul(kp, wk[:, DK * h : DK * (h + 1)], xkT)
            t = keep.tile([DK, TT], F32, tag=f"kT{h}")
            nc.scalar.activation(t, kp, AF.Identity,
                                 bias=bk_c[DK * h : DK * (h + 1)])
            kT[h] = t

        vb = {}
        for b in range(BPC):
            for jb, (j0, jl) in enumerate(IBLOCKS):
                vp = psp1.tile([128, 128], F32, tag="sm_ps")
                nc.tensor.matmul(vp[:jl, :], xvT[:, b * T + j0 : b * T + j0 + jl], wv)
                t = keep.tile([128, 128], BF16, tag=f"v{b}_{jb}")
                nc.vector.tensor_add(t[:jl, :], vp[:jl, :], bv_b[:jl, :])
                vb[(b, jb)] = t

        qaT = {}
        kaT = {}
        for hh in range(H_ABS):
            akT = transpose_in(absk[hh].flatten_outer_dims(), f"akT{hh}")
            pp = psp.tile([DK, TT], F32, tag="mm_ps")
            nc.tensor.matmul(pp, abs_w[("aqw", hh)], akT)
            pqT = prep.tile([DK, TT], F32, tag="pqT")
            nc.scalar.activation(pqT, pp, AF.Identity,
                                 bias=small_cols[("aqb_s", hh)], scale=SCALE)
            t = keep.tile([DK, TT], F32, tag=f"qaT{hh}")
            nc.vector.tensor_add(t, qT[hh], pqT)
            qaT[hh] = t

            pp2 = psp.tile([DK, TT], F32, tag="mm_ps")
            nc.tensor.matmul(pp2, abs_w[("akw", hh)], akT)
            pkT = prep.tile([DK, TT], F32, tag="pqT")
            nc.scalar.activation(pkT, pp2, AF.Identity,
                                 bias=small_cols[("akb", hh)])
            t = keep.tile([DK, TT], F32, tag=f"kaT{hh}")
            nc.vector.tensor_add(t, kT[hh], pkT)
            kaT[hh] = t

        mb = {}
        for b in range(BPC):
            for ib, (i0, il) in enumerate(IBLOCKS):
                mi = prep.tile([128, T], I32, tag="m_i32")
                small_dma(mi[:il, :], mask[b, 0, i0 : i0 + il, :])
                t = keep.tile([128, T], F32, tag=f"mb{b}_{ib}")
                nc.vector.tensor_scalar(t[:il, :], mi[:il, :], 1e9, -1e9,
                                        OP.mult, OP.add)
                mb[(b, ib)] = t

    # funnel: fires when every prep DMA has completed; gates the big stream
    funnel_t = consts.tile([1, 1], F32, tag="funnel")
    funnel = nc.vector.memset(funnel_t, 0.0)
    for di in prep_dmas:
        bass._add_dep_helper(funnel.ins, di.ins, True, "prep dma done")

    # ---------------- main phase ----------------
    stream = ctx.enter_context(tc.tile_pool(name="stream", bufs=5))
    wpool = ctx.enter_context(tc.tile_pool(name="wpool", bufs=3))
    tree = ctx.enter_context(tc.tile_pool(name="tree", bufs=2))
    s2pool = ctx.enter_context(tc.tile_pool(name="s2pool", bufs=2))
    sm = ctx.enter_context(tc.tile_pool(name="sm", bufs=2))
    ps_s1 = ctx.enter_context(tc.tile_pool(name="ps_s1", bufs=2, space="PSUM"))
    ps_tp = ctx.enter_context(tc.tile_pool(name="ps_tp", bufs=2, space="PSUM"))
    ps_tail = ctx.enter_context(tc.tile_pool(name="ps_tail", bufs=1, space="PSUM"))

    n_gated = [0]
    plan = chunk_engine_plan()
    chunk_idx = [0]

    def stream_chunk(hr, b, ib, jc0, s2t, ub):
        i0, il = IBLOCKS[ib]
        use_pool = plan[chunk_idx[0] % N_CHUNKS]
        chunk_idx[0] += 1
        eng = nc.gpsimd if use_pool else nc.vector
        wp = wpool
        tp = tree

        rk = stream.tile([128, JC, 128], BF16, tag="rk")
        # HWDGE stream: alternate the two hardware DGE rings
        dma_eng = nc.sync if chunk_idx[0] % 2 == 0 else nc.scalar
        dma_i = dma_eng.dma_start(
            rk[:il], relk[hr, b, i0 : i0 + il, jc0 : jc0 + JC, :]
        )
        if n_gated[0] < 12:
            bass._add_dep_helper(dma_i.ins, funnel.ins, True,
                                 "hold stream until prep loads done")
            n_gated[0] += 1
        w = wp.tile([128, JC, 128], BF16, tag="w")
        eng.tensor_tensor(
            w[:il], rk[:il],
            ub[:il, :].unsqueeze(1).broadcast_to([il, JC, 128]),
            op=OP.mult,
        )
        cur = w
        width = 64
        while width >= 2:
            nxt = tp.tile([128, JC, width], BF16, tag=f"L{width}")
            eng.tensor_add(
                nxt[:il], cur[:il, :, 0:width], cur[:il, :, width : 2 * width]
            )
            cur = nxt
            width //= 2
        eng.tensor_add(
            s2t[:il, jc0 : jc0 + JC], cur[:il, :, 0], cur[:il, :, 1]
        )

    for b in range(BPC):
        # stream both i-blocks interleaved: their DMAs hit disjoint
        # partition ranges -> more SDMA engines active concurrently
        s2 = {0: {}, 1: {}}
        for hr in range(H_REL):
            for ib in range(2):
                s2[ib][hr] = s2pool.tile([128, T], F32, tag=f"s2_{hr}_{ib}", name=f"s2_{hr}_{ib}")
            for jc0 in range(0, T, JC):
                for ib in range(2):
                    stream_chunk(hr, b, ib, jc0, s2[ib][hr], u_b[(hr, b, ib)])

        x_ps = {ib: ps_tail.tile([128, 128], F32, tag=f"x{ib}", name=f"x{ib}")
                for ib in range(2)}
        rcps = {}
        for ib, (i0, il) in enumerate(IBLOCKS):
            tsl = slice(b * T + i0, b * T + i0 + il)
            for h in range(H):
                is_rel = h >= H_ABS
                lhs = qaT[h] if not is_rel else qT[h]
                rhs = kaT[h] if not is_rel else kT[h]
                s1 = ps_s1.tile([128, T], F32, tag="s1")
                nc.tensor.matmul(s1[:il, :], lhs[:, tsl],
                                 rhs[:, b * T : (b + 1) * T])

                st = sm.tile([128, T], F32, tag="st")
                if is_rel:
                    hr = h - H_ABS
                    # st = (s1 + c) + s2, then + mask
                    nc.vector.scalar_tensor_tensor(
                        st[:il, :], s1[:il, :], c_sb[(hr, b, ib)][:il],
                        s2[ib][hr][:il, :], op0=OP.add, op1=OP.add)
                    nc.vector.tensor_add(st[:il, :], st[:il, :],
                                         mb[(b, ib)][:il, :])
                else:
                    nc.vector.tensor_add(st[:il, :], s1[:il, :],
                                         mb[(b, ib)][:il, :])

                nmax = sm.tile([128, 1], F32, tag="nmax")
                nc.vector.tensor_reduce(nmax[:il], st[:il, :], AX.X, OP.max,
                                        negate=True)
                # unnormalized p in bf16; 1/rsum folded into x copy later
                p = sm.tile([128, T], BF16, tag="p")
                rsum = sm.tile([128, 1], F32, tag="rsum")
                nc.scalar.activation(p[:il, :], st[:il, :], AF.Exp,
                                     bias=nmax[:il], accum_out=rsum[:il])
                rcp = sm.tile([128, 1], F32, tag=f"rcp{ib}_{h}")
                nc.vector.reciprocal(rcp[:il], rsum[:il])
                rcps[(ib, h)] = rcp

                hsl = slice(DK * h, DK * (h + 1))
                for jb, (j0, jl) in enumerate(IBLOCKS):
                    tp = ps_tp.tile([128, 128], BF16, tag="tp")
                    nc.tensor.transpose(tp[:jl, :il], p[:il, j0 : j0 + jl],
                                        identb[:il, :il])
                    pT = sm.tile([128, 128], BF16, tag="pT")
                    nc.scalar.copy(pT[:jl, :il], tp[:jl, :il])
                    nc.tensor.matmul(x_ps[ib][:il, hsl], pT[:jl, :il],
                                     vb[(b, jb)][:jl, hsl],
                                     start=(jb == 0), stop=(jb == 1))

        for ib, (i0, il) in enumerate(IBLOCKS):
            x_sb = sm.tile([128, 128], F32, tag="x_sb")
            for h in range(H):
                hsl = slice(DK * h, DK * (h + 1))
                nc.scalar.activation(x_sb[:il, hsl], x_ps[ib][:il, hsl],
                                     AF.Copy, scale=rcps[(ib, h)][:il])
            xT_ps = ps_tail.tile([128, 128], F32, tag="tail3")
            nc.tensor.transpose(xT_ps[:, :il], x_sb[:il, :], ident[:il, :il])
            xT_sb = sm.tile([128, 128], F32, tag="xT_sb")
            nc.scalar.copy(xT_sb[:, :il], xT_ps[:, :il])
            y_ps = ps_tail.tile([128, 128], F32, tag="tail3")
            nc.tensor.matmul(y_ps[:il, :], xT_sb[:, :il], wo)
            y_sb = sm.tile([128, 128], F32, tag="y_sb")
            nc.vector.tensor_add(y_sb[:il, :], y_ps[:il, :], bo_b[:il, :])
            nc.sync.dma_start(out[b, i0 : i0 + il, :], y_sb[:il, :])


def build_nc():
    nc = bacc.Bacc(trn_type="TRN2")
    io = {}
    io["query"] = nc.dram_tensor("query", [BPC, T, D], F32, kind="ExternalInput").ap()
    io["key"] = nc.dram_tensor("key", [BPC, T, D], F32, kind="ExternalInput").ap()
    io["value"] = nc.dram_tensor("value", [BPC, T, D], F32, kind="ExternalInput").ap()
    io["mask"] = nc.dram_tensor("mask", [BPC, 1, T, T], I32, kind="ExternalInput").ap()
    io["rel_kernel"] = nc.dram_tensor(
        "rel_kernel", [H_REL, BPC, T, T, D], BF16, kind="ExternalInput"
    ).ap()
    io["abs_kernel"] = nc.dram_tensor(
        "abs_kernel", [H_ABS, BPC, T, D], F32, kind="ExternalInput"
    ).ap()
    for nm, shape in [
        ("Wq", [D, D]), ("bq", [D]), ("Wk", [D, D]), ("bk", [D]),
        ("Wv", [D, D]), ("bv", [D]),
        ("abs_q_w", [H_ABS, D, DK]), ("abs_q_b", [H_ABS, DK]),
        ("abs_k_w", [H_ABS, D, DK]), ("abs_k_b", [H_ABS, DK]),
        ("rel_k_w", [H_REL, D, DK]), ("rel_k_b", [H_REL, DK]),
        ("rel_bias", [1, H_REL, 1, DK]),
        ("Wo", [D, D]), ("bo", [D]),
    ]:
        io[nm] = nc.dram_tensor(nm, shape, F32, kind="ExternalInput").ap()
    io["out"] = nc.dram_tensor("out", [BPC, T, D], F32, kind="ExternalOutput").ap()

    with tile.TileContext(nc) as tc:
        with ExitStack() as ctx:
            build_kernel(ctx, tc, io)
    nc.compile()
    return nc


_NC_CACHE = None


def _get_nc():
    global _NC_CACHE
    if _NC_CACHE is None:
        _NC_CACHE = build_nc()
    return _NC_CACHE


def make_in_maps(inputs):
    """Shard full inputs into per-core input maps."""
    f32 = np.float32
    weights = {
        nm: np.ascontiguousarray(np.asarray(inputs[nm], dtype=f32))
        for nm in ["Wq", "bq", "Wk", "bk", "Wv", "bv", "abs_q_w", "abs_q_b",
                   "abs_k_w", "abs_k_b", "rel_k_w", "rel_k_b", "rel_bias",
                   "Wo", "bo"]
    }
    query = np.asarray(inputs["query"], dtype=f32)
    key = np.asarray(inputs["key"], dtype=f32)
    value = np.asarray(inputs["value"], dtype=f32)
    mask = np.asarray(inputs["mask"], dtype=np.int32)
    import ml_dtypes
    relk = np.asarray(inputs["rel_kernel"], dtype=f32).astype(ml_dtypes.bfloat16)
    absk = np.asarray(inputs["abs_kernel"], dtype=f32)

    in_maps = []
    for c in range(N_CORES):
        bs = slice(c * BPC, (c + 1) * BPC)
        m = dict(weights)
        m["query"] = np.ascontiguousarray(query[bs])
        m["key"] = np.ascontiguousarray(key[bs])
        m["value"] = np.ascontiguousarray(value[bs])
        m["mask"] = np.ascontiguousarray(mask[bs])
        m["rel_kernel"] = np.ascontiguousarray(relk[:, bs])
        m["abs_kernel"] = np.ascontiguousarray(absk[:, bs])
        in_maps.append(m)
    return in_maps


def kernel(**inputs) -> np.ndarray:
    nc = _get_nc()
    in_maps = make_in_maps(inputs)
    res = run_bass_kernel_spmd(nc, in_maps, core_ids=list(range(N_CORES)))
    return np.concatenate([r["out"] for r in res.results], axis=0)


if __name__ == "__main__":
    nc = build_nc()
    print("built ok")
